# revision 3
# baseline (speedup 1.0000x reference)
"""HGCN forward on 8 TRN2 NeuronCores.

Strategy (graph/data parallel, per sharding hint):
- Nodes padded to 100352 = 8*12544 and sharded across cores (12544/core).
- Per-node math (hyperboloid linear/exp/log maps) in node-major [128,128]
  SBUF tiles; dense weights replicated; weight matmuls via PE transpose.
- hyp_agg: edges sorted by destination tile; per 128-edge chunk, gather
  xt[src] rows with indirect DMA from a replicated xt table (built each
  layer by AllGather of per-core shards), build a one-hot*weight matrix
  with a fused tensor_scalar (is_equal, mult) against an iota constant,
  and accumulate dst-tile aggregates on the TensorEngine in PSUM.
"""
import sys, types
import numpy as np

sys.path.insert(0, "/opt/trn_rl_repo")

# NTFF profile hook shim (antenv.axon_hooks is absent in this image).
if "antenv.axon_hooks" not in sys.modules:
    _m = types.ModuleType("antenv.axon_hooks")
    _hh = [None]
    _m.set_axon_ntff_profile_hook = lambda h: _hh.__setitem__(0, h)
    _m.get_axon_ntff_profile_hook = lambda: _hh[0]
    sys.modules["antenv.axon_hooks"] = _m
    try:
        from trn_agent_boot.trn_boot import _ntff_profile_via_ctypes
        _m.set_axon_ntff_profile_hook(_ntff_profile_via_ctypes("/opt/axon/libaxon_pjrt.so"))
    except Exception:
        pass

import concourse.bass as bass
import concourse.tile as tile
from concourse import bacc, mybir
import concourse.bass_utils as _bu
_bu.upload_artifacts = lambda d: "local://skipped"
from concourse.bass_utils import run_bass_kernel_spmd
from contextlib import ExitStack

F = np.float32
EPS = 1e-7
MIN = 1e-15
NC = 8
P = 128
DT = mybir.dt.float32


def _host_ub(b, c):
    # u_b = logmap0(proj(expmap0(proj_tan0(b), c), c), c), faithful f32.
    K = F(1.0 / c)
    sK = F(np.sqrt(K))
    y = b[1:].astype(F)
    yn = max(np.sqrt((y * y).sum(dtype=F)), F(MIN))
    th = min(yn / sK, F(15.0))
    sh = F(np.sinh(th))
    ch = F(np.cosh(th))
    hb_s = sK * sh * y / yn
    hb0 = F(np.sqrt(max(K + (hb_s * hb_s).sum(dtype=F), F(EPS))))
    thh = max(hb0 / sK, F(1.0 + EPS))
    ac = F(np.log(thh + np.sqrt(thh * thh - 1)))
    ybn = max(F(np.sqrt((hb_s * hb_s).sum(dtype=F))), F(MIN))
    u_s = sK * ac * hb_s / ybn
    out = np.zeros(b.shape[0], F)
    out[1:] = u_s
    return out


def _build(T, Kc, NPAD, out_d=64):
    """One SPMD program for all 8 cores. T node-tiles/core, Kc chunks/tile."""
    S = T * P
    nc = bacc.Bacc("TRN2", target_bir_lowering=False, debug=False, num_devices=NC)

    xpT = nc.dram_tensor("xpT", [T, P, P], DT, kind="ExternalInput")
    idx_d = nc.dram_tensor("idx", [T, P, Kc], mybir.dt.int32, kind="ExternalInput")
    meta_d = nc.dram_tensor("meta", [T, P, 2 * Kc], DT, kind="ExternalInput")
    consts = nc.dram_tensor("consts", [P, 896], DT, kind="ExternalInput")
    out_d_t = nc.dram_tensor("out", [S, out_d], DT, kind="ExternalOutput")

    xt1_sh = nc.dram_tensor("xt1_sh", [S, P], DT)
    xt1_full = nc.dram_tensor("xt1_full", [NPAD, P], DT, addr_space="Shared")
    xt2_sh = nc.dram_tensor("xt2_sh", [S, P], DT)
    xt2_full = nc.dram_tensor("xt2_full", [NPAD, P], DT, addr_space="Shared")

    sK = [F(np.sqrt(3.0)), F(np.sqrt(2.0)), F(1.0)]
    A = mybir.AluOpType

    with tile.TileContext(nc) as tc, ExitStack() as ctx:
        cp = ctx.enter_context(tc.tile_pool(name="consts", bufs=1))
        xpp = ctx.enter_context(tc.tile_pool(name="xp", bufs=3))
        gp = ctx.enter_context(tc.tile_pool(name="gath", bufs=2))
        mp = ctx.enter_context(tc.tile_pool(name="meta", bufs=2))
        ip = ctx.enter_context(tc.tile_pool(name="idx", bufs=2))
        wk = ctx.enter_context(tc.tile_pool(name="work", bufs=3))
        sc = ctx.enter_context(tc.tile_pool(name="scal", bufs=3))
        mtp = ctx.enter_context(tc.tile_pool(name="mt", bufs=3))
        pag = ctx.enter_context(tc.tile_pool(name="pag", bufs=2, space="PSUM"))
        pmv = ctx.enter_context(tc.tile_pool(name="pmv", bufs=2, space="PSUM"))
        ptr = ctx.enter_context(tc.tile_pool(name="ptr", bufs=2, space="PSUM"))

        ct = cp.tile([P, 896], DT)
        nc.sync.dma_start(out=ct[:], in_=consts[:])
        W1T = ct[:, 0:128]
        W2T = ct[:, 128:256]
        WlT = ct[:, 256:320]
        UB1 = ct[:, 320:448]
        UB2 = ct[:, 448:576]
        UBL = ct[:, 576:640]
        IDN = ct[:, 640:768]
        IOTA = ct[:, 768:896]

        _nsn = [0]

        def ns():  # fresh scalar tile
            _nsn[0] = (_nsn[0] + 1) % 40
            nm = "s" + str(_nsn[0])
            return sc.tile([P, 1], DT, tag=nm, name=nm)

        def expmap_mobius(mv_ps, UB, k, D):
            """expmap0+proj then mobius_add(+u_b)+proj at curvature index k.
            mv_ps: PSUM [P, D] (col0 junk). Returns (L tile [P,D], ln2 [P,1], L0 [P,1])."""
            sk = float(sK[k]); ik = 1.0 / sk; K = sk * sk
            scr = wk.tile([P, D], DT, tag="scr", name="scr")
            mn2 = ns()
            nc.scalar.activation(scr[:, 1:D], mv_ps[:, 1:D], mybir.ActivationFunctionType.Square, accum_out=mn2[:])
            mnr = ns(); nc.scalar.sqrt(mnr[:], mn2[:])
            mnc = ns(); nc.vector.tensor_scalar(mnc[:], mnr[:], MIN, None, A.max)
            thc = ns(); nc.vector.tensor_scalar(thc[:], mnc[:], ik, 15.0, A.mult, A.min)
            ea = ns(); nc.scalar.activation(ea[:], thc[:], mybir.ActivationFunctionType.Exp)
            eb = ns(); nc.scalar.activation(eb[:], thc[:], mybir.ActivationFunctionType.Exp, scale=-1.0)
            sh2 = ns(); nc.vector.tensor_tensor(sh2[:], ea[:], eb[:], A.subtract)
            ch2 = ns(); nc.vector.tensor_tensor(ch2[:], ea[:], eb[:], A.add)
            rmn = ns(); nc.vector.reciprocal(rmn[:], mnc[:])
            g1 = ns(); nc.vector.tensor_scalar(g1[:], sh2[:], rmn[:, :1], 0.5 * sk, A.mult, A.mult)
            x0v = ns(); nc.vector.tensor_scalar(x0v[:], ch2[:], 0.5 * sk, None, A.mult)
            r1 = wk.tile([P, D], DT, tag="r1", name="r1")
            nc.scalar.activation(r1[:, :D], mv_ps[:, :D], mybir.ActivationFunctionType.Copy, scale=g1[:, :1])
            nc.scalar.copy(r1[:, 0:1], x0v[:])
            yn = ns(); nc.vector.tensor_scalar(yn[:], g1[:], mnc[:, :1], MIN, A.mult, A.max)
            nc.vector.tensor_tensor(scr[:, 1:D], r1[:, 1:D], UB[:, 1:D], A.mult)
            d1 = ns(); nc.vector.tensor_reduce(d1[:], scr[:, 1:D], mybir.AxisListType.X, A.add)
            ryn = ns(); nc.vector.reciprocal(ryn[:], yn[:])
            alpha = ns(); nc.vector.tensor_scalar(alpha[:], d1[:], ryn[:, :1], ik, A.mult, A.mult)
            skx = ns(); nc.vector.tensor_scalar(skx[:], x0v[:], sk, -1.0, A.subtract, A.mult)
            t2 = ns(); nc.vector.tensor_tensor(t2[:], alpha[:], skx[:], A.mult)
            scal1 = ns(); nc.vector.tensor_tensor(scal1[:], t2[:], ryn[:], A.mult)
            t3 = wk.tile([P, D], DT, tag="t3", name="t3")
            nc.vector.tensor_scalar(t3[:, :D], r1[:, :D], scal1[:, :1], None, A.mult)
            res = wk.tile([P, D], DT, tag="res", name="res")
            nc.vector.tensor_tensor(res[:, :D], UB[:, :D], t3[:, :D], A.subtract)
            nc.vector.tensor_tensor(scr[:, 1:D], r1[:, 1:D], res[:, 1:D], A.mult)
            ux = ns(); nc.vector.tensor_reduce(ux[:], scr[:, 1:D], mybir.AxisListType.X, A.add)
            rx0 = ns(); nc.vector.reciprocal(rx0[:], x0v[:])
            v0 = ns(); nc.vector.tensor_tensor(v0[:], ux[:], rx0[:], A.mult)
            nc.scalar.copy(res[:, 0:1], v0[:])  # res is now v
            mdp = ns()
            nc.scalar.activation(scr[:, 1:D], res[:, 1:D], mybir.ActivationFunctionType.Square, accum_out=mdp[:])
            v0q = ns(); nc.vector.tensor_tensor(v0q[:], v0[:], v0[:], A.mult)
            md = ns(); nc.vector.tensor_tensor(md[:], mdp[:], v0q[:], A.subtract)
            mdc = ns(); nc.vector.tensor_scalar(mdc[:], md[:], EPS, None, A.max)
            nur = ns(); nc.scalar.sqrt(nur[:], mdc[:])
            th2 = ns(); nc.vector.tensor_scalar(th2[:], nur[:], 1e6, ik, A.min, A.mult)
            th2m = ns(); nc.vector.tensor_scalar(th2m[:], th2[:], MIN, None, A.max)
            th2c = ns(); nc.vector.tensor_scalar(th2c[:], th2m[:], 15.0, None, A.min)
            ea2 = ns(); nc.scalar.activation(ea2[:], th2c[:], mybir.ActivationFunctionType.Exp)
            eb2 = ns(); nc.scalar.activation(eb2[:], th2c[:], mybir.ActivationFunctionType.Exp, scale=-1.0)
            sh22 = ns(); nc.vector.tensor_tensor(sh22[:], ea2[:], eb2[:], A.subtract)
            ch22 = ns(); nc.vector.tensor_tensor(ch22[:], ea2[:], eb2[:], A.add)
            rt2 = ns(); nc.vector.reciprocal(rt2[:], th2m[:])
            s2 = ns(); nc.vector.tensor_scalar(s2[:], sh22[:], rt2[:, :1], 0.5, A.mult, A.mult)
            t4 = wk.tile([P, D], DT, tag="t4", name="t4")
            nc.vector.tensor_scalar(t4[:, :D], r1[:, :D], ch22[:, :1], 0.5, A.mult, A.mult)
            t5 = wk.tile([P, D], DT, tag="t5", name="t5")
            nc.scalar.activation(t5[:, :D], res[:, :D], mybir.ActivationFunctionType.Copy, scale=s2[:, :1])
            L = wk.tile([P, D], DT, tag="L", name="L")
            nc.vector.tensor_tensor(L[:, :D], t4[:, :D], t5[:, :D], A.add)
            ln2 = ns()
            nc.scalar.activation(scr[:, 1:D], L[:, 1:D], mybir.ActivationFunctionType.Square, accum_out=ln2[:])
            lnk = ns(); nc.vector.tensor_scalar(lnk[:], ln2[:], float(K), None, A.add)
            L0 = ns(); nc.scalar.sqrt(L0[:], lnk[:])
            nc.scalar.copy(L[:, 0:1], L0[:])
            return L, ln2, L0

        def logmap_xt(L, ln2, L0, k):
            sk = float(sK[k]); ik = 1.0 / sk
            ynr = ns(); nc.scalar.sqrt(ynr[:], ln2[:])
            ync = ns(); nc.vector.tensor_scalar(ync[:], ynr[:], MIN, None, A.max)
            thL = ns(); nc.vector.tensor_scalar(thL[:], L0[:], ik, 1.0 + EPS, A.mult, A.max)
            tq = ns(); nc.vector.tensor_tensor(tq[:], thL[:], thL[:], A.mult)
            tqm = ns(); nc.vector.tensor_scalar(tqm[:], tq[:], -1.0, None, A.add)
            sq = ns(); nc.scalar.sqrt(sq[:], tqm[:])
            ai = ns(); nc.vector.tensor_tensor(ai[:], thL[:], sq[:], A.add)
            ac = ns(); nc.scalar.activation(ac[:], ai[:], mybir.ActivationFunctionType.Ln)
            ry = ns(); nc.vector.reciprocal(ry[:], ync[:])
            fL = ns(); nc.vector.tensor_scalar(fL[:], ac[:], ry[:, :1], sk, A.mult, A.mult)
            xt = wk.tile([P, P], DT, tag="xt", name="xt")
            nc.scalar.activation(xt[:], L[:], mybir.ActivationFunctionType.Copy, scale=fL[:, :1])
            return xt

        def agg_tile(t, table):
            idx_t = ip.tile([P, Kc], mybir.dt.int32, name="idx_t")
            nc.sync.dma_start(out=idx_t[:], in_=idx_d[t])
            met = mp.tile([P, 2 * Kc], DT, name="met")
            nc.sync.dma_start(out=met[:], in_=meta_d[t])
            G = gp.tile([P, Kc * P], DT, tag="G", name="G")
            for kk in range(Kc):
                nc.gpsimd.indirect_dma_start(
                    out=G[:, kk * P:(kk + 1) * P], out_offset=None,
                    in_=table[:],
                    in_offset=bass.IndirectOffsetOnAxis(ap=idx_t[:, kk:kk + 1], axis=0),
                )
            agg = pag.tile([P, P], DT, space="PSUM", name="aggp")
            for kk in range(Kc):
                Mt = mtp.tile([P, P], DT, tag="Mt", name="Mt")
                nc.vector.tensor_scalar(Mt[:], IOTA[:], met[:, kk:kk + 1], met[:, Kc + kk:Kc + kk + 1],
                                        A.is_equal, A.mult)
                nc.tensor.matmul(agg[:], lhsT=Mt[:], rhs=G[:, kk * P:(kk + 1) * P],
                                 start=(kk == 0), stop=(kk == Kc - 1))
            return agg

        def post_agg(agg, kin, kout):
            ski, iki = float(sK[kin]), 1.0 / float(sK[kin])
            sko, iko = float(sK[kout]), 1.0 / float(sK[kout])
            scr2 = wk.tile([P, P], DT, tag="scr2", name="scr2")
            an2 = ns()
            nc.scalar.activation(scr2[:, 1:P], agg[:, 1:P], mybir.ActivationFunctionType.Square, accum_out=an2[:])
            anr = ns(); nc.scalar.sqrt(anr[:], an2[:])
            anc = ns(); nc.vector.tensor_scalar(anc[:], anr[:], MIN, None, A.max)
            th3 = ns(); nc.vector.tensor_scalar(th3[:], anc[:], iki, 15.0, A.mult, A.min)
            ran = ns(); nc.vector.reciprocal(ran[:], anc[:])
            h3 = ns(); nc.vector.tensor_scalar(h3[:], th3[:], ran[:, :1], ski, A.mult, A.mult)
            xt2 = wk.tile([P, P], DT, tag="xt2", name="xt2")
            nc.vector.tensor_scalar(xt2[:], agg[:], h3[:, :1], 0.0, A.mult, A.max)
            y42 = ns()
            nc.scalar.activation(scr2[:, 1:P], xt2[:, 1:P], mybir.ActivationFunctionType.Square, accum_out=y42[:])
            y4r = ns(); nc.scalar.sqrt(y4r[:], y42[:])
            y4c = ns(); nc.vector.tensor_scalar(y4c[:], y4r[:], MIN, None, A.max)
            th4 = ns(); nc.vector.tensor_scalar(th4[:], y4c[:], iko, 15.0, A.mult, A.min)
            r4 = ns(); nc.vector.reciprocal(r4[:], y4c[:])
            m5 = ns(); nc.vector.tensor_scalar(m5[:], th4[:], r4[:, :1], sko, A.mult, A.mult)
            lg = wk.tile([P, P], DT, tag="lg", name="lg")
            nc.scalar.activation(lg[:], xt2[:], mybir.ActivationFunctionType.Copy, scale=m5[:, :1])
            return lg

        def lin_mm(lg, WT, D):
            trp = ptr.tile([P, P], DT, space="PSUM", name="trp")
            nc.tensor.transpose(trp[:], lg[:], IDN[:])
            lgT = wk.tile([P, P], DT, tag="lgT", name="lgT")
            nc.vector.tensor_copy(lgT[:], trp[:])
            mv = pmv.tile([P, D], DT, space="PSUM", tag="mv", name="mvp")
            nc.tensor.matmul(mv[:], lhsT=lgT[:], rhs=WT[:, :D], start=True, stop=True)
            return mv

        # ---- Phase A ----
        for t in range(T):
            xt_in = xpp.tile([P, P], DT)
            nc.sync.dma_start(out=xt_in[:], in_=xpT[t])
            mv = pmv.tile([P, P], DT, space="PSUM", tag="mv")
            nc.tensor.matmul(mv[:], lhsT=xt_in[:], rhs=W1T[:], start=True, stop=True)
            L, ln2, L0 = expmap_mobius(mv, UB1, 0, P)
            xt = logmap_xt(L, ln2, L0, 0)
            nc.sync.dma_start(out=xt1_sh[t * P:(t + 1) * P, :], in_=xt[:])
        nc.gpsimd.collective_compute("AllGather", A.bypass, replica_groups=[list(range(NC))],
                                     ins=[xt1_sh[:]], outs=[xt1_full[:]])
        # ---- Phase B ----
        for t in range(T):
            agg = agg_tile(t, xt1_full)
            lg2 = post_agg(agg, 0, 1)
            mv2 = lin_mm(lg2, W2T, P)
            L2, ln2b, L0b = expmap_mobius(mv2, UB2, 1, P)
            xt2t = logmap_xt(L2, ln2b, L0b, 1)
            nc.sync.dma_start(out=xt2_sh[t * P:(t + 1) * P, :], in_=xt2t[:])
        nc.gpsimd.collective_compute("AllGather", A.bypass, replica_groups=[list(range(NC))],
                                     ins=[xt2_sh[:]], outs=[xt2_full[:]])
        # ---- Phase C ----
        for t in range(T):
            agg = agg_tile(t, xt2_full)
            lg3 = post_agg(agg, 1, 2)
            mv3 = lin_mm(lg3, WlT, out_d)
            Lf, _, _ = expmap_mobius(mv3, UBL, 2, out_d)
            nc.sync.dma_start(out=out_d_t[t * P:(t + 1) * P, :], in_=Lf[:])

    nc.compile()
    return nc


def _prep(x, edge_index, edge_weight, W1, b1, W2, b2, Wl, bl, NPAD):
    N = x.shape[0]
    S = NPAD // NC
    T = S // P
    GT = NPAD // P
    src = edge_index[0].astype(np.int64)
    dst = edge_index[1].astype(np.int64)
    w = edge_weight.astype(F)
    order = np.argsort(dst, kind="stable")
    srcs, dsts, ws = src[order], dst[order], w[order]
    gt = dsts >> 7
    cnt = np.bincount(gt, minlength=GT)
    Kc = max(1, int(np.ceil(cnt.max() / P)))
    CAP = Kc * P
    starts = np.zeros(GT, np.int64)
    starts[1:] = np.cumsum(cnt)[:-1]
    pos = np.arange(len(srcs)) - starts[gt]
    pad_src = np.zeros((GT, CAP), np.int32)
    pad_rel = np.zeros((GT, CAP), F)
    pad_w = np.zeros((GT, CAP), F)
    pad_src[gt, pos] = srcs
    pad_rel[gt, pos] = (dsts - (gt << 7)).astype(F)
    pad_w[gt, pos] = ws

    # layouts per core: idx [T,P,Kc] with idx[t,p,k]=edge (t,k*128+p); meta [T,P,2Kc]
    idx_all = pad_src.reshape(GT, Kc, P).transpose(0, 2, 1)          # [GT,P,Kc]
    rel_all = pad_rel.reshape(GT, Kc, P).transpose(0, 2, 1)
    w_all = pad_w.reshape(GT, Kc, P).transpose(0, 2, 1)
    meta_all = np.concatenate([rel_all, w_all], axis=2)              # [GT,P,2Kc]

    xp = np.zeros((NPAD, P), F)
    xp[:N, 1:] = x
    Tc = T
    xpT = xp.reshape(NPAD // P, P, P).transpose(0, 2, 1)             # [GT,P,P] transposed tiles

    def ZW(Wm):
        We = Wm.astype(F).copy()
        We[:, 0] = 0
        return np.ascontiguousarray(We.T)

    ub1 = _host_ub(b1.astype(F), 1.0 / 3.0)
    ub2 = _host_ub(b2.astype(F), 0.5)
    ubl = _host_ub(bl.astype(F), 1.0)
    consts = np.zeros((P, 896), F)
    consts[:, 0:128] = ZW(W1)
    consts[:, 128:256] = ZW(W2)
    consts[:, 256:320] = ZW(Wl)
    consts[:, 320:448] = np.tile(ub1, (P, 1))
    consts[:, 448:576] = np.tile(ub2, (P, 1))
    consts[:, 576:640] = np.tile(ubl, (P, 1))
    consts[:, 640:768] = np.eye(P, dtype=F)
    consts[:, 768:896] = np.tile(np.arange(P, dtype=F), (P, 1))

    in_maps = []
    for c in range(NC):
        in_maps.append({
            "xpT": np.ascontiguousarray(xpT[c * Tc:(c + 1) * Tc]),
            "idx": np.ascontiguousarray(idx_all[c * Tc:(c + 1) * Tc]),
            "meta": np.ascontiguousarray(meta_all[c * Tc:(c + 1) * Tc]),
            "consts": consts,
        })
    return in_maps, T, Kc


_CACHE = {}


def kernel(x, edge_index, edge_weight, W1, b1, W2, b2, Wl, bl, trace=False):
    N = x.shape[0]
    NPAD = ((N + NC * P - 1) // (NC * P)) * NC * P
    in_maps, T, Kc = _prep(x, edge_index, edge_weight, W1, b1, W2, b2, Wl, bl, NPAD)
    key = (T, Kc, NPAD)
    if key not in _CACHE:
        _CACHE[key] = _build(T, Kc, NPAD, 64)
    nc = _CACHE[key]
    r = run_bass_kernel_spmd(nc, in_maps, list(range(NC)), trace=trace)
    out = np.concatenate([r.results[c]["out"] for c in range(NC)], axis=0)[:N]
    kernel.last_exec_ns = r.exec_time_ns
    return out.astype(np.float32)


kernel.last_exec_ns = None
